# revision 41
# baseline (speedup 1.0000x reference)
"""OFA attention (dense_transformer) on 8 Trainium2 NeuronCores.

Sharding: heads split over cores (core c owns heads {2c, 2c+1}, both batches).

Per-core Bass/Tile program (build_attention_nc):
  phase 1 : QT/KT/VT = W_c @ hs.T (transposed projections; SCALING folded into
            Wq, c_attn folded into Wv on host; bias-add fused into the PSUM
            drain on ScalarE)
  phase 1b: V natural = PE-transpose(VT), packed [V_A | 1 | V_B | 1] bf16
  phase 2 : per (batch, 512-token t-block), streaming 128-row s-tiles:
              ST(s,t) = K Q^T            (row-tiled K=64 matmuls, 2 heads)
              E  = exp(ST)               (ScalarE, PSUM -> SBUF bf16)
              E *= exp(bias).T           (DVE; exp(bias [+mask]) is precomputed
                                          on the host in bf16 and DMA'd already
                                          transposed -- no PE bias matmuls, and
                                          half the HBM bias traffic of f32)
              [O.T ; sums] += [V|1].T@E  (PV matmul also yields softmax sums)
            epilogue: sums rows -> reciprocal (DVE) -> replicated to all 128
            partitions by a tiny K=2 selector matmul (PE) -> O.T normalized by
            one DVE multiply -> single contract-128 out-projection (heads
            summed inside the matmul) -> PSUM drained bf16 -> DMA out.
Host: partial outputs summed over cores + bo (the "all-reduce" of out_proj).
"""
import sys

for _p in ("/opt/trn_rl_repo",):
    if _p not in sys.path:
        sys.path.append(_p)

import numpy as np

import concourse.bass as bass
import concourse.tile as tile
from concourse import mybir
from concourse.masks import make_identity
from concourse.bass_utils import run_bass_kernel_spmd

F32 = mybir.dt.float32
BF16 = mybir.dt.bfloat16

B, T, E, NH, D = 2, 2048, 1024, 16, 64
N_CORES = 8
HPC = NH // N_CORES
DH = HPC * D
SCALING = float(D * 2.0) ** -0.5


def _waitfix(nc, limit=1):
    """This walrus build accepts at most ONE sync-wait per instruction.
    Hoist excess sem-waits onto inserted single-wait NoOps."""
    n_fixed = 0
    for bb in nc.m.functions[0].blocks:
        i = 0
        insts = bb.instructions
        while i < len(insts):
            inst = insts[i]
            si = inst.sync_info
            if si and si.on_wait and len(si.on_wait) > limit:
                extra = si.on_wait[limit:]
                si.on_wait = si.on_wait[:limit]
                for k, w in enumerate(extra):
                    nop = mybir.InstNoOp(
                        name=f"{inst.name}-waitfix{k}",
                        engine=inst.engine,
                        sync_info=mybir.SyncInfo(on_wait=[w], on_update=[]),
                        bass_nofuse=True,
                    )
                    nc.register_instruction(nop, overwrite=True)
                    insts.insert(i, nop)
                    i += 1
                n_fixed += 1
            i += 1
    return n_fixed


def build_attention_nc(B=2, T=2048, E=1024, HPC=2, D=64,
                       T_BLOCK=512, PROJ_BLOCK=512):
    """Build the per-core Bass program. Returns nc."""
    S = T
    PROJ_BLOCK = min(PROJ_BLOCK, T)
    TOK = B * T
    DH = HPC * D                      # 128
    assert DH == 128 and D == 64
    NE = E // 128                     # e-tiles
    NST = S // 128                    # s-tiles per batch
    NTB = T // T_BLOCK                # t-blocks per batch
    NJ = T_BLOCK // 128               # t-subtiles per block
    NPB = TOK // PROJ_BLOCK           # proj token blocks

    nc = bass.Bass()

    hsT = nc.declare_dram_parameter("hsT", [E, TOK], BF16, isOutput=False)
    # weights pre-shuffled on host to the SBUF layout [p, e-tile, dh] so the
    # load is 128 contiguous 2KB descriptors instead of 1024 256B ones
    wqT = nc.declare_dram_parameter("wqT", [128, E // 128, DH], BF16, isOutput=False)
    wkT = nc.declare_dram_parameter("wkT", [128, E // 128, DH], BF16, isOutput=False)
    wvT = nc.declare_dram_parameter("wvT", [128, E // 128, DH], BF16, isOutput=False)
    bq = nc.declare_dram_parameter("bq", [DH, 1], F32, isOutput=False)
    bk = nc.declare_dram_parameter("bk", [DH, 1], F32, isOutput=False)
    bv = nc.declare_dram_parameter("bv", [DH, 1], F32, isOutput=False)
    woT = nc.declare_dram_parameter("woT", [DH, E], BF16, isOutput=False)
    # exp(bias [+ mask]) pre-transposed on the host:
    # xb[(b*NTB+tb)*HPC*2 + a*2 + jh, p, j*T_BLOCK + t] =
    #     exp(bias)[b, head a, t = tb*T_BLOCK + t, s = (jh*8 + j)*128 + p]
    NXB = B * NTB * HPC * 2
    xb_in = nc.declare_dram_parameter(
        "xb", [NXB, 128, (NST // 2) * T_BLOCK], BF16, isOutput=False)
    out_partial = nc.declare_dram_parameter("out", [TOK, E], BF16, isOutput=True)

    with tile.TileContext(nc) as tc:
        from contextlib import ExitStack
        with ExitStack() as ctx:
            consts = ctx.enter_context(tc.tile_pool(name="consts", bufs=1))
            persist = ctx.enter_context(tc.tile_pool(name="persist", bufs=1))

            i_bf = consts.tile([128, 128], BF16, tag="i_bf")
            make_identity(nc, i_bf[:])
            # weights: (E, DH) -> (128, NE, DH), bf16 (scalar-engine HWDGE
            # queue: keeps the sync queue free for the hsT strips)
            w_sb = {}
            for name, src in (("wq", wqT), ("wk", wkT), ("wv", wvT)):
                t = consts.tile([128, NE, DH], BF16, tag=name)
                nc.scalar.dma_start(out=t[:], in_=src[:, :, :])
                w_sb[name] = t
            wo_sb = consts.tile([128, E], BF16, tag="wo")
            nc.scalar.dma_start(out=wo_sb[:], in_=woT[:, :])
            b_sb = {}
            for name, src in (("bq", bq), ("bk", bk), ("bv", bv)):
                t = consts.tile([128, 1], F32, tag=name)
                nc.scalar.dma_start(out=t[:], in_=src[:, :])
                b_sb[name] = t
            gate_sb = consts.tile([1, 1], BF16, tag="gate")
            # head-selectors for replicating the sums rows to partitions:
            # rank-1 matmuls ones_a^T @ s_a put s_0 on partitions 0:64 and
            # s_1 on 64:128 (partition-0-only tiles; partition starts must
            # be quad-aligned so a [2,128] selector is not expressible)
            ones_sel = []
            for a in range(HPC):
                oa = consts.tile([1, 128], BF16, tag=f"ones{a}", name=f"ones{a}")
                nc.vector.memset(oa[:, :], 0.0)
                nc.vector.memset(oa[0:1, a * D:(a + 1) * D], 1.0)
                ones_sel.append(oa)

            # persistent activations (QT/KT bf16; VT bf16 for the PE transpose)
            QTb = [persist.tile([128, T], BF16, tag=f"QT{bb}", name=f"QT{bb}")
                   for bb in range(B)]
            KTb = [persist.tile([128, T], BF16, tag=f"KT{bb}", name=f"KT{bb}")
                   for bb in range(B)]
            VTb = [persist.tile([128, T], BF16, tag=f"VT{bb}", name=f"VT{bb}")
                   for bb in range(B)]
            V_sbb = [persist.tile([128, T // 128, 256], BF16, tag=f"V_sb{bb}",
                                  name=f"V_sb{bb}") for bb in range(B)]

            def init_v_sb(V_sb):
                # cols 0:D and 128:128+D are overwritten by the transposed V;
                # only the ones columns and the padding tails need init
                nc.vector.memset(V_sb[:, :, D:128], 0.0)
                nc.vector.memset(V_sb[:, :, 128 + D:256], 0.0)
                nc.vector.memset(V_sb[:, :, D:D + 1], 1.0)
                nc.vector.memset(V_sb[:, :, 128 + D:128 + D + 1], 1.0)

            # bias pool is allocated BEFORE the phase-1 pools so prefetch DMAs
            # can run during the projections.  Tags cycle per (head, s-half);
            # bufs=3 gives ~1.5 t-blocks of lookahead (96 KiB).
            bias_pool = ctx.enter_context(tc.tile_pool(name="xbias", bufs=2))
            bias_tiles = {}  # (b, tb, a, jh) -> tile, issued ~2 tbs ahead

            def issue_bias(b, tb):
                for a in range(HPC):
                    for jh in range(2):
                        bt = bias_pool.tile([128, NST // 2, T_BLOCK], BF16,
                                            tag=f"xb{a}{jh}",
                                            name=f"xb{b}_{tb}_{a}_{jh}")
                        idx = ((b * NTB + tb) * HPC + a) * 2 + jh
                        nc.gpsimd.dma_start(
                            out=bt[:],
                            in_=xb_in[idx].rearrange("p (j t) -> p j t",
                                                     t=T_BLOCK))
                        bias_tiles[(b, tb, a, jh)] = bt

            # ---------------- phase 1: projections + V transpose ----------
            with tc.tile_pool(name="hst", bufs=1) as hst_pool, \
                 tc.tile_pool(name="proj_ps", bufs=3, space="PSUM") as proj_ps, \
                 tc.tile_pool(name="vtr_ps", bufs=4, space="PSUM") as vtr_ps:
                # all 16 strips issued upfront (per-batch tags), split over
                # two DMA queues: trigger issue rate (~1.1us each) bounds how
                # fast the first proj block can start
                hstrips = {}
                for bb2 in range(B):
                    for e in range(NE):
                        h = hst_pool.tile([128, T], BF16, tag=f"hst{bb2}_{e}",
                                          name=f"hst{bb2}_{e}")
                        eng = nc.sync if e % 2 == 0 else nc.gpsimd
                        eng.dma_start(
                            out=h[:], in_=hsT[e * 128:(e + 1) * 128,
                                              bb2 * T:(bb2 + 1) * T])
                        hstrips[(bb2, e)] = h
                for V_sb in V_sbb:
                    init_v_sb(V_sb)
                # gate each batch's bias prefetch behind its hsT strips so the
                # SWDGE bias stream cannot starve them on the shared DMA
                # engines; (0,0)/(0,1) issued in consumption order
                for bb2 in range(B):
                    nc.gpsimd.tensor_copy(out=gate_sb[0:1, 0:1],
                                          in_=hstrips[(bb2, NE - 1)][0:1, 0:1])
                    issue_bias(0, bb2)
                for bb2 in range(B):
                    for pbl in range(T // PROJ_BLOCK):
                        tloc = pbl * PROJ_BLOCK
                        for name, dstl in (("wq", QTb), ("wk", KTb), ("wv", VTb)):
                            ps = proj_ps.tile([128, PROJ_BLOCK], F32, tag="proj",
                                              name=f"pps{bb2}_{pbl}_{name}")
                            for e in range(NE):
                                nc.tensor.matmul(ps[:], w_sb[name][:, e, :],
                                                 hstrips[(bb2, e)][:, tloc:tloc + PROJ_BLOCK],
                                                 start=(e == 0), stop=(e == NE - 1))
                            nc.scalar.activation(
                                out=dstl[bb2][:, tloc:tloc + PROJ_BLOCK], in_=ps[:],
                                func=mybir.ActivationFunctionType.Identity,
                                bias=b_sb["b" + name[1]][:], scale=1.0)
                        # V transpose for the four s-tiles this block drained
                        # (spreads the PE transposes + DVE copy-backs)
                        for st in range(pbl * 4, pbl * 4 + 4):
                            ps = vtr_ps.tile([128, 128], BF16, tag="vtr",
                                             name=f"vtr{bb2}_{st}")
                            nc.tensor.transpose(
                                ps[:], VTb[bb2][:, st * 128:(st + 1) * 128],
                                i_bf[:])
                            nc.vector.tensor_copy(out=V_sbb[bb2][:, st, 0:D],
                                                  in_=ps[:, 0:D])
                            nc.vector.tensor_copy(
                                out=V_sbb[bb2][:, st, 128:128 + D],
                                in_=ps[:, D:2 * D])

            # ---------------- phase 2: attention ----------------
            with tc.tile_pool(name="e_sb", bufs=8) as e_pool, \
                 tc.tile_pool(name="otn_sb", bufs=2) as otn_pool, \
                 tc.tile_pool(name="srow", bufs=2) as srow_pool, \
                 tc.tile_pool(name="rrep", bufs=2) as rrep_pool, \
                 tc.tile_pool(name="osb", bufs=3) as out_pool, \
                 tc.tile_pool(name="st_ps", bufs=3, space="PSUM") as st_ps, \
                 tc.tile_pool(name="ot_ps", bufs=2, space="PSUM") as ot_ps:

                def emit_tail(tl):
                    """Drain the previous block: its two trailing PV groups,
                    then O.T rows + bf16 sums rows.  Returns the epi pend."""
                    emit_pv_p, pendq_p, ots_p, tglob_p = tl
                    for pend_pv in pendq_p:
                        emit_pv_p(pend_pv)
                    otn = otn_pool.tile([128, T_BLOCK], BF16, tag="otn",
                                        name=f"otn{tglob_p}")
                    s2 = []
                    # sums rows first: they feed the PE replication matmul
                    for a in range(HPC):
                        sa = srow_pool.tile([1, T_BLOCK], BF16, tag=f"s{a}",
                                            name=f"srow{tglob_p}_{a}")
                        nc.vector.tensor_copy(out=sa[0:1, :],
                                              in_=ots_p[a][D:D + 1, :])
                        s2.append(sa)
                    for a in range(HPC):
                        nc.vector.tensor_copy(out=otn[a * D:(a + 1) * D, :],
                                              in_=ots_p[a][0:D, :])
                    return (otn, s2, tglob_p)

                def emit_epi_rep(pend):
                    """Replicate the sums rows to all partitions with two
                    rank-1 selector matmuls into bank 0 of the first
                    out-projection PSUM tile (no extra PSUM bank), then one
                    wide reciprocal and the O.T normalization on DVE."""
                    otn_p, s2_p, tglob_p = pend
                    wo_t0 = st_ps.tile([128, 2, T_BLOCK], F32, tag="st",
                                       name=f"wot0{tglob_p}")
                    for a in range(HPC):
                        nc.tensor.matmul(wo_t0[:, 0, :], ones_sel[a][0:1, :],
                                         s2_p[a][0:1, :],
                                         start=(a == 0), stop=(a == HPC - 1))
                    rrep = rrep_pool.tile([128, T_BLOCK], F32, tag="rrep",
                                          name=f"rrep{tglob_p}")
                    nc.vector.reciprocal(rrep[:], wo_t0[:, 0, :])
                    nc.vector.tensor_mul(out=otn_p[:], in0=otn_p[:], in1=rrep[:])
                    return wo_t0

                def emit_epi_wo(pend, wo_t0):
                    """Out-projection (heads summed in-matmul), drain, DMA."""
                    otn_p, s2_p, tglob_p = pend
                    for k in range(NJ):
                        wo_ps = wo_t0 if k == 0 else st_ps.tile(
                            [128, 2, T_BLOCK], F32, tag="st",
                            name=f"wops{tglob_p}_{k}")
                        for n in range(2):
                            nc.tensor.matmul(
                                wo_ps[:, n, :],
                                otn_p[:, k * 128:(k + 1) * 128],
                                wo_sb[:, n * T_BLOCK:(n + 1) * T_BLOCK],
                                start=True, stop=True)
                        os_t = out_pool.tile([128, E], BF16, tag="osb",
                                             name=f"osb{tglob_p}_{k}")
                        nc.scalar.activation(
                            out=os_t[:], in_=wo_ps[:],
                            func=mybir.ActivationFunctionType.Copy)
                        nc.sync.dma_start(
                            out=out_partial[tglob_p + k * 128:
                                            tglob_p + (k + 1) * 128, :],
                            in_=os_t[:])

                tail = None
                pending = None
                pend_wo_t0 = None
                for b in range(B):
                    for tb in range(NTB):
                        tglob = b * T + tb * T_BLOCK
                        # prefetch bias two blocks ahead ((0,0)/(0,1) were
                        # issued during phase 1)
                        nxt = b * NTB + tb + 2
                        if nxt < B * NTB:
                            issue_bias(nxt // NTB, nxt % NTB)

                        ots = [ot_ps.tile([128, T_BLOCK], F32, tag="ot",
                                          name=f"ot{b}_{tb}_{a}")
                               for a in range(HPC)]

                        def emit_pv(pend_pv, ots=ots, b=b):
                            for a, e_ap, pst in pend_pv:
                                nc.tensor.matmul(
                                    ots[a][:],
                                    V_sbb[b][:, pst, a * 128:a * 128 + 128],
                                    e_ap,
                                    start=(pst == 0), stop=(pst == NST - 1))

                        pendq = []
                        for sp in range(NST // 2):
                            if sp == 2 and pending is not None:
                                pend_wo_t0 = emit_epi_rep(pending)
                            if sp == 5 and pending is not None:
                                emit_epi_wo(pending, pend_wo_t0)
                                pending = None
                            stp2 = []
                            for a in range(HPC):
                                stp2.append(st_ps.tile([128, 2, T_BLOCK], F32,
                                                       tag="st",
                                                       name=f"st{b}_{tb}_{sp}_{a}"))
                            for half in range(2):
                                st = sp * 2 + half
                                for a in range(HPC):
                                    r0 = a * D
                                    nc.tensor.matmul(
                                        stp2[a][:, half, :],
                                        KTb[b][r0:r0 + D, st * 128:st * 128 + 128],
                                        QTb[b][r0:r0 + D,
                                               tb * T_BLOCK:tb * T_BLOCK + T_BLOCK],
                                        start=True, stop=True)
                            # tail of the previous block: emitted after sp0's
                            # scores (PE overlap) but before sp0's multiplies
                            # (keeps the sums-row copies early in the DVE queue)
                            if sp == 0 and tail is not None:
                                pending = emit_tail(tail)
                                tail = None
                            if len(pendq) >= 2:
                                emit_pv(pendq.pop(0))
                            jh, j0 = sp // 4, (sp % 4) * 2
                            pend_pv = []
                            for a in range(HPC):
                                e_t = e_pool.tile([128, 2, T_BLOCK], BF16, tag="et",
                                                  name=f"et{b}_{tb}_{sp}_{a}")
                                nc.scalar.activation(
                                    out=e_t[:], in_=stp2[a][:],
                                    func=mybir.ActivationFunctionType.Exp)
                                nc.vector.tensor_mul(
                                    out=e_t[:], in0=e_t[:],
                                    in1=bias_tiles[(b, tb, a, jh)][:, j0:j0 + 2, :])
                                for half in range(2):
                                    pend_pv.append((a, e_t[:, half, :],
                                                    sp * 2 + half))
                            pendq.append(pend_pv)
                        # the two trailing PV groups + drains carry into the
                        # next block so the PE never waits on them here
                        tail = (emit_pv, pendq, ots, tglob)

                pending = emit_tail(tail)
                wo_t0 = emit_epi_rep(pending)
                emit_epi_wo(pending, wo_t0)
    _waitfix(nc)
    return nc


# ---------------- host-side prep ----------------

def shard_inputs(hidden_states, attn_bias, attention_mask, Wq, bq, Wk, bk, Wv, bv,
                 Wo, bo, c_attn, n_cores=8, scaling=None):
    """Build per-core input maps. Returns (in_maps, with_mask)."""
    import ml_dtypes
    bf16 = ml_dtypes.bfloat16
    B, T, E = hidden_states.shape
    NH = c_attn.shape[0]
    D = E // NH
    HPC = NH // n_cores
    DH = HPC * D
    T_BLOCK = 512
    NTB = T // T_BLOCK
    S = T

    with_mask = bool(np.any(attention_mask))
    hsT = np.ascontiguousarray(hidden_states.reshape(B * T, E).T).astype(bf16)
    bias4 = attn_bias.reshape(B, NH, T, S)
    if with_mask:
        bias4 = bias4 + attention_mask.reshape(B, 1, T, S)

    if scaling is None:
        scaling = float(D * 2.0) ** -0.5

    in_maps = []
    for c in range(n_cores):
        r0 = c * DH
        sl = slice(r0, r0 + DH)
        hsl = slice(c * HPC, (c + 1) * HPC)
        cvec = np.repeat(c_attn[c * HPC:(c + 1) * HPC], D)
        # exp(bias).T in bf16, tiled [idx, p, j*T_BLOCK + t]
        xbT = np.exp(bias4[:, hsl].transpose(0, 1, 3, 2))     # [B, HPC, S, T]
        xb = xbT.reshape(B, HPC, 2, S // 256, 128, NTB, T_BLOCK)
        xb = xb.transpose(0, 5, 1, 2, 4, 3, 6)  # [B, NTB, HPC, 2, 128, j, t]
        xb = np.ascontiguousarray(xb).astype(bf16).reshape(
            B * NTB * HPC * 2, 128, (S // 256) * T_BLOCK)
        def wshuf(wT):  # [E, DH] -> [128, E//128, DH] (SBUF layout)
            return np.ascontiguousarray(
                wT.reshape(E // 128, 128, DH).transpose(1, 0, 2)).astype(bf16)

        m = {
            "hsT": hsT,
            "wqT": wshuf((Wq[sl] * scaling).T),
            "wkT": wshuf(Wk[sl].T),
            "wvT": wshuf((Wv[sl] * cvec[:, None]).T),
            "bq": np.ascontiguousarray((bq[sl] * scaling)[:, None]).astype(np.float32),
            "bk": np.ascontiguousarray(bk[sl][:, None]).astype(np.float32),
            "bv": np.ascontiguousarray((bv[sl] * cvec)[:, None]).astype(np.float32),
            "woT": np.ascontiguousarray(Wo[:, sl].T).astype(bf16),
            "xb": xb,
        }
        in_maps.append(m)
    return in_maps, False


_NC_CACHE = {}


def run_spmd(in_maps, with_mask, **kwargs):
    if with_mask not in _NC_CACHE:
        _NC_CACHE[with_mask] = build_attention_nc(B=B, T=T, E=E, HPC=HPC, D=D)
    nc = _NC_CACHE[with_mask]
    return run_bass_kernel_spmd(nc, in_maps, list(range(N_CORES)), **kwargs)


def kernel(hidden_states, attn_bias, attention_mask, Wq, bq, Wk, bk, Wv, bv,
           Wo, bo, c_attn):
    args = [np.asarray(a, dtype=np.float32) for a in
            (hidden_states, attn_bias, attention_mask, Wq, bq, Wk, bk, Wv, bv,
             Wo, bo, c_attn)]
    (hidden_states, attn_bias, attention_mask, Wq, bq, Wk, bk, Wv, bv,
     Wo, bo, c_attn) = args
    in_maps, with_mask = shard_inputs(hidden_states, attn_bias, attention_mask,
                                      Wq, bq, Wk, bk, Wv, bv, Wo, bo, c_attn,
                                      n_cores=N_CORES, scaling=SCALING)
    res = run_spmd(in_maps, with_mask)
    out = np.zeros((B * T, E), np.float32)
    for r in res.results:
        out += r["out"]
    out += bo[None, :]
    return out.reshape(B, T, E).astype(np.float32)
